# revision 11
# baseline (speedup 1.0000x reference)
"""DINO loss kernel for 8 Trainium2 NeuronCores.

Math (per reference):
    pt  = softmax((vt - center) / 0.04)                       [512, K]
    ps  = log_softmax(vs / 0.1 + 1e-20)                       [1536, K]
    loss = mean over (c, i, j) of -sum_k pt[c,i,k] * ps[c,j,k]
with chunks c of 2 teacher rows / 6 student rows (only first 5 used).

Since sum_k pt = 1 (the 1e-20 terms cancel exactly):
    -pt . ps = log(S_j) - 10 * D[i,j] / Z_i
where a_i = exp(25*(vt_i - center) - 150)  (constant shift is safe for
N(0,1)-scale logits), Z_i = sum_k a_i[k], D[i,j] = sum_k a_i[k] vs_j[k],
S_j = sum_k exp(10 vs_j[k]).

Device (data-parallel, 32 chunks per core; K split 128 partitions x 512):
    - the Scalar (ACT) engine is the bottleneck: every element of vs and
      vt goes through one exp at 1 elem/cycle/lane. Everything else is
      scheduled around keeping ACT 100% busy from ~8us to the end:
      a warmup exp pulls the table load off the critical path, teacher
      f-chunks are finely graded at the start, the first student subtiles
      are exp'd in row chunks as their DMA lands, and student subtile
      sizes taper at the end so the trailing DVE tree work is tiny.
    - D and Z via PSUM-accumulated matmuls: stationary = teacher exp
      slice [128, 64], moving = student slice + ones row [128, 161]
      (column 160 accumulates Z_i for free). Even/odd k-slices go to the
      two PE column halves via tile_position; host adds the two halves.
      Teacher exps run ~2 subtiles ahead of the matmul need so PE drains
      early.
    - S_j row sums: per-subtile log-tree pair-adds on VectorE; subtile
      results are folded mid-stream on GpSimd so the final combine after
      the last exp is one small tree + two adds.
Host does the final tiny reduction in float64.
"""

import os
import sys

import numpy as np

try:
    import ml_dtypes
except ImportError:  # pragma: no cover
    ml_dtypes = None

for _p in ("/opt/trn_rl_repo", "/root/.axon_site/_ro/trn_rl_repo"):
    if os.path.isdir(_p) and _p not in sys.path:
        sys.path.insert(0, _p)

K = 65536
P = 128
F = K // P          # 512 free elems per partition per row
N_CORES = 8
N_VIEWS = 5
S_CHUNK = 256       # total chunks
CPC = S_CHUNK // N_CORES   # 32 chunks per core
TR = 2 * CPC        # 64 teacher rows per core
SR = N_VIEWS * CPC  # 160 student rows per core
SCALE_T = 25.0      # 1 / 0.04
SCALE_S = 10.0      # 1 / 0.1
SHIFT_T = 150.0     # 25 * 6.0; exp(25*x - 150) never overflows for
                    # |x| <~ 9.5 and keeps Z in fp32 normal range for
                    # gaussian logits (row max ~4.5 -> Z ~ e^-40).

# student subtile widths (f-cols): big in the middle, tapered at the end
SIZES = [32] * 14 + [16, 16, 16, 8, 8]
OFFS = [sum(SIZES[:i]) for i in range(len(SIZES))]
NS = len(SIZES)
assert sum(SIZES) == F
MAXSZ = max(SIZES)
# per-subtile row chunks for DMA + exp (first subtiles stream in by rows
# so ACT starts before the whole tile lands)
ROWCH = {0: [0, 40, 80, 120, SR + 1], 1: [0, 80, SR + 1]}

# teacher f-chunks, finely graded at the start
TCH = [(0, 4), (4, 12), (12, 28), (28, 60), (60, 124), (124, 188),
       (188, 256), (256, 320), (320, 384), (384, 448), (448, 512)]
TLOOK = 64          # teacher exp emission lookahead (f-cols)

_CACHE = {}
LAST_EXEC_NS = None


def _build():
    import concourse.bacc as bacc
    import concourse.mybir as mybir
    import concourse.tile as tile

    bf16 = mybir.dt.bfloat16
    f32 = mybir.dt.float32

    nc = bacc.Bacc("TRN2", target_bir_lowering=False, debug=False,
                   num_devices=N_CORES)

    vt_in = nc.dram_tensor("vt", [P, F, TR], bf16, kind="ExternalInput")
    # per partition: concat over subtiles of [SR+1, sz] blocks (j-major)
    vs_in = nc.dram_tensor("vs", [P, (SR + 1) * F], bf16,
                           kind="ExternalInput")
    # cols [0:SR+1] = D|Z psum copy, [SR+1:2*SR+1] = sfin
    out_t = nc.dram_tensor("out", [P, 2 * SR + 1], f32, kind="ExternalOutput")

    from concourse.tile import add_dep_helper

    EXP = mybir.ActivationFunctionType.Exp
    ADD = mybir.AluOpType.add

    with tile.TileContext(nc) as tc:
        with (
            tc.tile_pool(name="ap", bufs=1) as ap_pool,
            tc.tile_pool(name="vsp", bufs=4) as vs_pool,
            tc.tile_pool(name="evsp", bufs=3) as evs_pool,
            tc.tile_pool(name="outp", bufs=1) as out_pool,
            tc.tile_pool(name="psum", bufs=1, space="PSUM") as psum_pool,
        ):
            # teacher exp bias, written by memset (no DMA)
            bias_t = ap_pool.tile([P, 1], f32, tag="biast")
            nc.vector.memset(bias_t[:], -SHIFT_T)

            # Warmup: pull the ~1.3us EXP table load (plus ACT pipeline
            # spin-up) off the critical path; depends only on a memset.
            warm_t = ap_pool.tile([P, 1], f32, tag="warm")
            nc.vector.memset(warm_t[:], 0.0)
            warm = nc.scalar.activation(out=warm_t[:], in_=warm_t[:],
                                        func=EXP, bias=0.0, scale=1.0)

            a_t = ap_pool.tile([P, F, TR], bf16, tag="teacher")
            act_chain = []

            def chain_act(h):
                # add_dep_helper(a, b) == "a waits on b"
                if act_chain:
                    add_dep_helper(h.ins, act_chain[-1].ins, sync=False,
                                   reason="act consumption order")
                act_chain.append(h)

            chain_act(warm)

            vec_chain = []

            def chain_vec(h):
                if vec_chain:
                    add_dep_helper(h.ins, vec_chain[-1].ins, sync=False,
                                   reason="dve emission order")
                vec_chain.append(h)
                return h

            # [0:64]  <- even k-slices (PE col half 0)
            # [64:128] <- odd k-slices (PE col half 1); host adds halves.
            dots_ps = psum_pool.tile([P, SR + 1], f32, tag="dots")
            # cols 0..7: per-subtile sums (folded round-robin on GpSimd),
            # cols 8..9: scratch ping-pong for subtiles >= 8
            sreds = ap_pool.tile([P, SR, 10], f32, tag="sreds")

            def s_tree(evs_ap, rows, n, out_col):
                # log-tree pair-add of n dense bf16 cols -> f32 column.
                stree = vs_pool.tile([P, SR, MAXSZ // 2], bf16, tag="stree")
                st = stree[:, rows, 0:n // 2]
                chain_vec(nc.vector.tensor_tensor(
                    out=st, in0=evs_ap[:, :, 0:n // 2],
                    in1=evs_ap[:, :, n // 2:n], op=ADD))
                w = n // 4
                while w >= 1:
                    dst = stree[:, rows, 0:w] if w > 1 else out_col
                    chain_vec(nc.vector.tensor_tensor(
                        out=dst, in0=stree[:, rows, 0:w],
                        in1=stree[:, rows, w:2 * w], op=ADD))
                    w //= 2

            sb_out = out_pool.tile([P, 2 * SR + 1], f32, tag="oall")
            sfin = sb_out[:, SR + 1:2 * SR + 1]

            tex_handles = []   # (start_f, activation handle)
            waited_chunks = 0  # chunks the PE stream is already gated on
            prev_mm = None     # pin PE order: start=True must run first
            copied = False
            for s in range(NS):
                off, sz = OFFS[s], SIZES[s]
                # emit teacher chunks with lookahead so PE drains early
                while len(tex_handles) < len(TCH) and (
                        TCH[len(tex_handles)][0] < min(F, off + sz + TLOOK)):
                    fr = slice(*TCH[len(tex_handles)])
                    nc.sync.dma_start(out=a_t[:, fr, :], in_=vt_in[:, fr, :])
                    tex = nc.scalar.activation(
                        out=a_t[:, fr, :], in_=a_t[:, fr, :],
                        func=EXP, bias=bias_t[:], scale=SCALE_T)
                    chain_act(tex)
                    tex_handles.append((fr.start, tex))

                vs_t = vs_pool.tile([P, SR + 1, MAXSZ], bf16, tag="vs")
                evs_t = evs_pool.tile([P, SR, MAXSZ], bf16, tag="evs")
                base = (SR + 1) * off
                rch = ROWCH.get(s, [0, SR + 1])
                for r0, r1 in zip(rch[:-1], rch[1:]):
                    nc.sync.dma_start(
                        out=vs_t[:, r0:r1, 0:sz],
                        in_=vs_in[:, base + r0 * sz:base + r1 * sz])
                    er1 = min(r1, SR)
                    chain_act(nc.scalar.activation(
                        out=evs_t[:, r0:er1, 0:sz],
                        in_=vs_t[:, r0:er1, 0:sz],
                        func=EXP, bias=0.0, scale=SCALE_S))
                    if s < 8:
                        col = sreds[:, r0:er1, s]
                        s_tree(evs_t[:, r0:er1, 0:sz], slice(r0, er1),
                               sz, col)
                if s >= 8:
                    scr = 8 + (s % 2)
                    s_tree(evs_t[:, 0:SR, 0:sz], slice(0, SR), sz,
                           sreds[:, :, scr])
                    if s <= 16:
                        # fold into base column on GpSimd (off DVE)
                        dstc = (s - 8) % 8
                        nc.gpsimd.tensor_tensor(
                            out=sreds[:, :, dstc], in0=sreds[:, :, dstc],
                            in1=sreds[:, :, scr], op=ADD)
                    if s == 16:
                        # everything except s17/s18 is now in cols 0..7:
                        # combine into sfin while the last exps run
                        chain_vec(nc.vector.tensor_tensor(
                            out=sreds[:, :, 0:4], in0=sreds[:, :, 0:4],
                            in1=sreds[:, :, 4:8], op=ADD))
                        chain_vec(nc.vector.tensor_tensor(
                            out=sreds[:, :, 0:2], in0=sreds[:, :, 0:2],
                            in1=sreds[:, :, 2:4], op=ADD))
                        chain_vec(nc.vector.tensor_tensor(
                            out=sfin, in0=sreds[:, :, 0],
                            in1=sreds[:, :, 1], op=ADD))

                # D (cols 0..159) and Z (col 160) accumulate together.
                for lf in range(sz):
                    f = off + lf
                    half = f % 2
                    mm = nc.tensor.matmul(
                        dots_ps[64 * half:64 * half + TR, :],
                        a_t[:, f, :], vs_t[:, :, lf],
                        start=(f == half), stop=(f >= F - 2),
                        tile_position=(0, 64 * half))
                    # PSUM accumulation is only correct in program order
                    # (start=True clears the bank) -- forbid reordering.
                    if prev_mm is not None:
                        add_dep_helper(mm.ins, prev_mm.ins, sync=False,
                                       reason="psum accumulation order")
                    prev_mm = mm
                    # explicitly gate PE on the teacher-exp chunks this
                    # subtile's weights come from (the weights-operand
                    # RAW dep is not reliably tracked); PE is in-order,
                    # so one edge per newly needed chunk suffices.
                    while (waited_chunks < len(tex_handles)
                           and tex_handles[waited_chunks][0] < off + sz):
                        add_dep_helper(mm.ins,
                                       tex_handles[waited_chunks][1].ins,
                                       reason="weights ready")
                        waited_chunks += 1

            # ACT is idle after its exps while DVE drains trees: use it
            # for the PSUM->SBUF copy of D|Z (waits on the last matmul)
            chain_act(nc.scalar.copy(sb_out[:, 0:SR + 1], dots_ps[:]))
            nc.sync.dma_start(out=out_t[:, 0:SR + 1],
                              in_=sb_out[:, 0:SR + 1])
            # fold the last two subtile sums (scratch cols 9=s17, 8=s18)
            chain_vec(nc.vector.tensor_tensor(out=sfin, in0=sfin,
                                              in1=sreds[:, :, 9], op=ADD))
            chain_vec(nc.vector.tensor_tensor(out=sfin, in0=sfin,
                                              in1=sreds[:, :, 8], op=ADD))
            nc.sync.dma_start(out=out_t[:, SR + 1:2 * SR + 1], in_=sfin)

    nc.compile()
    return nc


def _get_nc():
    if "nc" not in _CACHE:
        _CACHE["nc"] = _build()
    return _CACHE["nc"]


def kernel(vs: np.ndarray, vt: np.ndarray, center: np.ndarray) -> np.ndarray:
    global LAST_EXEC_NS
    from concourse.bass_utils import run_bass_kernel_spmd

    bf = ml_dtypes.bfloat16
    vs = np.asarray(vs, dtype=np.float32)
    vt = np.asarray(vt, dtype=np.float32)
    center = np.asarray(center, dtype=np.float32)

    # Drop the unused 6th student view, center the teacher.
    vs_used = np.ascontiguousarray(
        vs.reshape(S_CHUNK, N_VIEWS + 1, K)[:, :N_VIEWS, :]
    ).reshape(S_CHUNK * N_VIEWS, K).astype(bf)
    vt_c = (vt - center).astype(bf)

    in_maps = []
    for d in range(N_CORES):
        vt_d = vt_c[TR * d:TR * (d + 1)]                     # [TR, K]
        # device layout: vt_dev[p, f, r] = vt_d[r, p*F + f]  (f-major so
        # matmul weight columns are contiguous in SBUF)
        vt_dev = np.ascontiguousarray(
            vt_d.reshape(TR, P, F).transpose(1, 2, 0))
        vs_d = vs_used[SR * d:SR * (d + 1)]                  # [SR, K]
        vs_p = vs_d.reshape(SR, P, F).transpose(1, 0, 2)     # [P, SR, F]
        # per partition: concat over subtiles of [SR+1, sz] j-major
        # blocks, with an all-ones row j=SR (accumulates Z in the matmul)
        vs_dev = np.empty((P, (SR + 1) * F), dtype=bf)
        for s in range(NS):
            off, sz = OFFS[s], SIZES[s]
            tmp = np.empty((P, SR + 1, sz), dtype=bf)
            tmp[:, :SR] = vs_p[:, :, off:off + sz]
            tmp[:, SR] = bf(1.0)
            b = (SR + 1) * off
            vs_dev[:, b:b + (SR + 1) * sz] = tmp.reshape(P, -1)
        in_maps.append({"vt": vt_dev, "vs": vs_dev})

    nc = _get_nc()
    trace = os.environ.get("BASS_DINO_TRACE", "0") == "1"
    res = run_bass_kernel_spmd(nc, in_maps, list(range(N_CORES)), trace=trace)
    LAST_EXEC_NS = res.exec_time_ns

    total = 0.0
    for d in range(N_CORES):
        out = res.results[d]["out"]
        DZ = out[:, :SR + 1].astype(np.float64)              # [P, SR+1]
        DZ = DZ[:TR] + DZ[TR:]                               # even + odd halves
        D, Z = DZ[:, :SR], DZ[:, SR]
        S = out[:, SR + 1:].astype(np.float64).sum(axis=0)   # [SR]
        lse = np.log(S)                                      # [SR]
        Dn = D * (SCALE_S / Z)[:, None]                      # [TR, SR]
        blk = Dn.reshape(CPC, 2, CPC, N_VIEWS)
        d_sum = blk[np.arange(CPC), :, np.arange(CPC), :].sum()
        total += 2.0 * lse.sum() - d_sum
    loss = total / (S_CHUNK * 2 * N_VIEWS)
    return np.asarray(loss, dtype=np.float32)


# revision 15
# speedup vs baseline: 1.2677x; 1.2677x over previous
"""DINO loss kernel for 8 Trainium2 NeuronCores.

Math (per reference):
    pt  = softmax((vt - center) / 0.04)                       [512, K]
    ps  = log_softmax(vs / 0.1 + 1e-20)                       [1536, K]
    loss = mean over (c, i, j) of -sum_k pt[c,i,k] * ps[c,j,k]
with chunks c of 2 teacher rows / 6 student rows (only first 5 used).

Since sum_k pt = 1 (the 1e-20 terms cancel exactly):
    -pt . ps = log(S_j) - 10 * D[i,j] / Z_i
where a_i = exp(25*(vt_i - center) - 150)  (constant shift is safe for
N(0,1)-scale logits), Z_i = sum_k a_i[k], D[i,j] = sum_k a_i[k] vs_j[k],
S_j = sum_k exp(10 vs_j[k]).

Device (data-parallel, 32 chunks per core; K split 128 partitions x 512):
    - the Scalar (ACT) engine is the bottleneck: every element of vs and
      vt goes through one exp at 1 elem/cycle/lane. Everything else is
      scheduled around keeping ACT 100% busy from ~8us to the end:
      a warmup exp pulls the table load off the critical path, teacher
      f-chunks are finely graded at the start, the first student subtiles
      are exp'd in row chunks as their DMA lands, and student subtile
      sizes taper at the end so the trailing DVE tree work is tiny.
    - D and Z via PSUM-accumulated matmuls: stationary = teacher exp
      slice [128, 64], moving = student slice + ones row [128, 161]
      (column 160 accumulates Z_i for free). Even/odd k-slices go to the
      two PE column halves via tile_position; host adds the two halves.
      Teacher exps run ~2 subtiles ahead of the matmul need so PE drains
      early.
    - S_j row sums: per-subtile log-tree pair-adds on VectorE; subtile
      results are folded mid-stream on GpSimd so the final combine after
      the last exp is one small tree + two adds.
Host does the final tiny reduction in float64.
"""

import os
import sys

import numpy as np

try:
    import ml_dtypes
except ImportError:  # pragma: no cover
    ml_dtypes = None

for _p in ("/opt/trn_rl_repo", "/root/.axon_site/_ro/trn_rl_repo"):
    if os.path.isdir(_p) and _p not in sys.path:
        sys.path.insert(0, _p)

K = 65536
P = 128
F = K // P          # 512 free elems per partition per row
N_CORES = 8
N_VIEWS = 5
S_CHUNK = 256       # total chunks
CPC = S_CHUNK // N_CORES   # 32 chunks per core
TR = 2 * CPC        # 64 teacher rows per core
SR = N_VIEWS * CPC  # 160 student rows per core
SCALE_T = 25.0      # 1 / 0.04
SCALE_S = 10.0      # 1 / 0.1
SHIFT_T = 150.0     # 25 * 6.0; exp(25*x - 150) never overflows for
                    # |x| <~ 9.5 and keeps Z in fp32 normal range for
                    # gaussian logits (row max ~4.5 -> Z ~ e^-40).

# student subtile widths (f-cols): big in the middle, tapered at the end
SIZES = [32] * 14 + [16, 16, 16, 8, 8]
OFFS = [sum(SIZES[:i]) for i in range(len(SIZES))]
NS = len(SIZES)
assert sum(SIZES) == F
MAXSZ = max(SIZES)
# per-subtile row chunks for DMA + exp (first subtiles stream in by rows
# so ACT starts before the whole tile lands)
ROWCH = {0: [0, 40, 80, 120, SR + 1], 1: [0, 80, SR + 1]}

# teacher f-chunks, finely graded at the start
TCH = [(0, 4), (4, 12), (12, 28), (28, 60), (60, 124), (124, 188),
       (188, 256), (256, 320), (320, 384), (384, 448), (448, 512)]
TLOOK = 64          # teacher exp emission lookahead (f-cols)

_CACHE = {}
LAST_EXEC_NS = None


def _build():
    import concourse.bacc as bacc
    import concourse.mybir as mybir
    import concourse.tile as tile

    bf16 = mybir.dt.bfloat16
    f32 = mybir.dt.float32

    nc = bacc.Bacc("TRN2", target_bir_lowering=False, debug=False,
                   num_devices=N_CORES)

    vt_in = nc.dram_tensor("vt", [P, F, TR], bf16, kind="ExternalInput")
    # per partition: concat over subtiles of [SR+1, sz] blocks (j-major)
    vs_in = nc.dram_tensor("vs", [P, (SR + 1) * F], bf16,
                           kind="ExternalInput")
    # cols [0:SR+1] = D|Z psum copy, [SR+1:2*SR+1] = sfin
    out_t = nc.dram_tensor("out", [P, 2 * SR + 1], f32, kind="ExternalOutput")

    from concourse.tile import add_dep_helper

    EXP = mybir.ActivationFunctionType.Exp
    ADD = mybir.AluOpType.add

    with tile.TileContext(nc) as tc:
        with (
            tc.tile_pool(name="ap", bufs=1) as ap_pool,
            tc.tile_pool(name="vsp", bufs=4) as vs_pool,
            tc.tile_pool(name="evsp", bufs=3) as evs_pool,
            tc.tile_pool(name="outp", bufs=1) as out_pool,
            tc.tile_pool(name="psum", bufs=1, space="PSUM") as psum_pool,
        ):
            # teacher exp bias, written by memset (no DMA)
            bias_t = ap_pool.tile([P, 1], f32, tag="biast")
            nc.vector.memset(bias_t[:], -SHIFT_T)

            # Warmup: pull the ~1.3us EXP table load (plus ACT pipeline
            # spin-up) off the critical path; depends only on a memset.
            warm_t = ap_pool.tile([P, 1], f32, tag="warm")
            nc.vector.memset(warm_t[:], 0.0)
            warm = nc.scalar.activation(out=warm_t[:], in_=warm_t[:],
                                        func=EXP, bias=0.0, scale=1.0)

            a_t = ap_pool.tile([P, F, TR], bf16, tag="teacher")
            act_chain = []

            def chain_act(h):
                # add_dep_helper(a, b) == "a waits on b"
                if act_chain:
                    add_dep_helper(h.ins, act_chain[-1].ins, sync=False,
                                   reason="act consumption order")
                act_chain.append(h)

            chain_act(warm)

            vec_chain = []

            def chain_vec(h):
                if vec_chain:
                    add_dep_helper(h.ins, vec_chain[-1].ins, sync=False,
                                   reason="dve emission order")
                vec_chain.append(h)
                return h

            # [0:64]  <- even k-slices (PE col half 0)
            # [64:128] <- odd k-slices (PE col half 1); host adds halves.
            dots_ps = psum_pool.tile([P, SR + 1], f32, tag="dots")
            # cols 0..7: base sums (subtiles 0..7 direct; 8..14 folded in
            # on GpSimd); cols 8..18: private per-subtile columns for
            # subtiles 8..18 (private = no WAR serialization)
            sreds = ap_pool.tile([P, SR, 19], f32, tag="sreds")

            def s_tree(evs_ap, rows, n, out_col):
                # log-tree pair-add of n dense bf16 cols -> f32 column.
                stree = vs_pool.tile([P, SR, n // 2], bf16, tag="stree",
                                     bufs=2)
                st = stree[:, rows, :]
                chain_vec(nc.vector.tensor_tensor(
                    out=st, in0=evs_ap[:, :, 0:n // 2],
                    in1=evs_ap[:, :, n // 2:n], op=ADD))
                w = n // 4
                while w >= 1:
                    dst = stree[:, rows, 0:w] if w > 1 else out_col
                    chain_vec(nc.vector.tensor_tensor(
                        out=dst, in0=stree[:, rows, 0:w],
                        in1=stree[:, rows, w:2 * w], op=ADD))
                    w //= 2

            sb_out = out_pool.tile([P, 2 * SR + 1], f32, tag="oall")
            sfin = sb_out[:, SR + 1:2 * SR + 1]

            tex_handles = []   # (start_f, activation handle)
            waited_chunks = 0  # chunks the PE stream is already gated on
            prev_mm = None     # pin PE order: start=True must run first
            for s in range(NS):
                off, sz = OFFS[s], SIZES[s]
                # emit teacher chunks with lookahead so PE drains early
                while len(tex_handles) < len(TCH) and (
                        TCH[len(tex_handles)][0] < min(F, off + sz + TLOOK)):
                    fr = slice(*TCH[len(tex_handles)])
                    nc.sync.dma_start(out=a_t[:, fr, :], in_=vt_in[:, fr, :])
                    tex = nc.scalar.activation(
                        out=a_t[:, fr, :], in_=a_t[:, fr, :],
                        func=EXP, bias=bias_t[:], scale=SCALE_T)
                    chain_act(tex)
                    tex_handles.append((fr.start, tex))

                vs_t = vs_pool.tile([P, SR + 1, sz], bf16, tag="vs")
                evs_t = evs_pool.tile([P, SR, sz], bf16, tag="evs")
                base = (SR + 1) * off
                rch = ROWCH.get(s, [0, SR + 1])
                for r0, r1 in zip(rch[:-1], rch[1:]):
                    nc.sync.dma_start(
                        out=vs_t[:, r0:r1, :],
                        in_=vs_in[:, base + r0 * sz:base + r1 * sz])
                    er1 = min(r1, SR)
                    chain_act(nc.scalar.activation(
                        out=evs_t[:, r0:er1, :],
                        in_=vs_t[:, r0:er1, :],
                        func=EXP, bias=0.0, scale=SCALE_S))
                    s_tree(evs_t[:, r0:er1, :], slice(r0, er1), sz,
                           sreds[:, r0:er1, s])
                if 8 <= s <= 14:
                    # fold private col into base col on GpSimd (off DVE)
                    nc.gpsimd.tensor_tensor(
                        out=sreds[:, :, s - 8], in0=sreds[:, :, s - 8],
                        in1=sreds[:, :, s], op=ADD)
                if s == 15:
                    # base cols 0..7 complete (needs GP folds <= s14):
                    # combine them while the tail subtiles run
                    chain_vec(nc.vector.tensor_tensor(
                        out=sreds[:, :, 0:4], in0=sreds[:, :, 0:4],
                        in1=sreds[:, :, 4:8], op=ADD))
                    chain_vec(nc.vector.tensor_tensor(
                        out=sreds[:, :, 0:2], in0=sreds[:, :, 0:2],
                        in1=sreds[:, :, 2:4], op=ADD))
                    chain_vec(nc.vector.tensor_tensor(
                        out=sreds[:, :, 0], in0=sreds[:, :, 0],
                        in1=sreds[:, :, 1], op=ADD))
                if s >= 16:
                    # fold tail cols into col 15 as they finish
                    chain_vec(nc.vector.tensor_tensor(
                        out=sreds[:, :, 15], in0=sreds[:, :, 15],
                        in1=sreds[:, :, s], op=ADD))

                # D (cols 0..159) and Z (col 160) accumulate together.
                for lf in range(sz):
                    f = off + lf
                    half = f % 2
                    mm = nc.tensor.matmul(
                        dots_ps[64 * half:64 * half + TR, :],
                        a_t[:, f, :], vs_t[:, :, lf],
                        start=(f == half), stop=(f >= F - 2),
                        tile_position=(0, 64 * half))
                    # PSUM accumulation is only correct in program order
                    # (start=True clears the bank) -- forbid reordering.
                    if prev_mm is not None:
                        add_dep_helper(mm.ins, prev_mm.ins, sync=False,
                                       reason="psum accumulation order")
                    prev_mm = mm
                    # explicitly gate PE on the teacher-exp chunks this
                    # subtile's weights come from (the weights-operand
                    # RAW dep is not reliably tracked); PE is in-order,
                    # so one edge per newly needed chunk suffices.
                    while (waited_chunks < len(tex_handles)
                           and tex_handles[waited_chunks][0] < off + sz):
                        add_dep_helper(mm.ins,
                                       tex_handles[waited_chunks][1].ins,
                                       reason="weights ready")
                        waited_chunks += 1

            # ACT is idle after its exps while DVE drains trees: use it
            # for the PSUM->SBUF copy of D|Z (waits on the last matmul)
            chain_act(nc.scalar.copy(sb_out[:, 0:SR + 1], dots_ps[:]))
            nc.sync.dma_start(out=out_t[:, 0:SR + 1],
                              in_=sb_out[:, 0:SR + 1])
            # final: sfin = (base combine) + (tail combine)
            chain_vec(nc.vector.tensor_tensor(out=sfin, in0=sreds[:, :, 0],
                                              in1=sreds[:, :, 15], op=ADD))
            nc.sync.dma_start(out=out_t[:, SR + 1:2 * SR + 1], in_=sfin)

    nc.compile()
    return nc


def _get_nc():
    if "nc" not in _CACHE:
        _CACHE["nc"] = _build()
    return _CACHE["nc"]


def kernel(vs: np.ndarray, vt: np.ndarray, center: np.ndarray) -> np.ndarray:
    global LAST_EXEC_NS
    from concourse.bass_utils import run_bass_kernel_spmd

    bf = ml_dtypes.bfloat16
    vs = np.asarray(vs, dtype=np.float32)
    vt = np.asarray(vt, dtype=np.float32)
    center = np.asarray(center, dtype=np.float32)

    # Drop the unused 6th student view, center the teacher.
    vs_used = np.ascontiguousarray(
        vs.reshape(S_CHUNK, N_VIEWS + 1, K)[:, :N_VIEWS, :]
    ).reshape(S_CHUNK * N_VIEWS, K).astype(bf)
    vt_c = (vt - center).astype(bf)

    in_maps = []
    for d in range(N_CORES):
        vt_d = vt_c[TR * d:TR * (d + 1)]                     # [TR, K]
        # device layout: vt_dev[p, f, r] = vt_d[r, p*F + f]  (f-major so
        # matmul weight columns are contiguous in SBUF)
        vt_dev = np.ascontiguousarray(
            vt_d.reshape(TR, P, F).transpose(1, 2, 0))
        vs_d = vs_used[SR * d:SR * (d + 1)]                  # [SR, K]
        vs_p = vs_d.reshape(SR, P, F).transpose(1, 0, 2)     # [P, SR, F]
        # per partition: concat over subtiles of [SR+1, sz] j-major
        # blocks, with an all-ones row j=SR (accumulates Z in the matmul)
        vs_dev = np.empty((P, (SR + 1) * F), dtype=bf)
        for s in range(NS):
            off, sz = OFFS[s], SIZES[s]
            tmp = np.empty((P, SR + 1, sz), dtype=bf)
            tmp[:, :SR] = vs_p[:, :, off:off + sz]
            tmp[:, SR] = bf(1.0)
            b = (SR + 1) * off
            vs_dev[:, b:b + (SR + 1) * sz] = tmp.reshape(P, -1)
        in_maps.append({"vt": vt_dev, "vs": vs_dev})

    nc = _get_nc()
    trace = os.environ.get("BASS_DINO_TRACE", "0") == "1"
    res = run_bass_kernel_spmd(nc, in_maps, list(range(N_CORES)), trace=trace)
    LAST_EXEC_NS = res.exec_time_ns

    total = 0.0
    for d in range(N_CORES):
        out = res.results[d]["out"]
        DZ = out[:, :SR + 1].astype(np.float64)              # [P, SR+1]
        DZ = DZ[:TR] + DZ[TR:]                               # even + odd halves
        D, Z = DZ[:, :SR], DZ[:, SR]
        S = out[:, SR + 1:].astype(np.float64).sum(axis=0)   # [SR]
        lse = np.log(S)                                      # [SR]
        Dn = D * (SCALE_S / Z)[:, None]                      # [TR, SR]
        blk = Dn.reshape(CPC, 2, CPC, N_VIEWS)
        d_sum = blk[np.arange(CPC), :, np.arange(CPC), :].sum()
        total += 2.0 * lse.sum() - d_sum
    loss = total / (S_CHUNK * 2 * N_VIEWS)
    return np.asarray(loss, dtype=np.float32)
